# Initial kernel scaffold
#
"""4-layer GCN (message passing) on 8 Trainium2 NeuronCores.

Layer: x1 = A_norm @ (x @ W) + b ; x = relu(x1). A_norm = D^-1/2 (A+I) D^-1/2.

Strategy (dst-sharded graph parallel, per the sharding hint):
  - Nodes are partitioned across the 8 cores by destination (6250 each).
  - Each core redundantly computes the full dense transform H = X @ W
    (cheap on TensorE) and writes H as a row-major bf16 gather table in
    its local DRAM.
  - Each core gathers H[src] rows for the edges pointing into its own
    dst shard with the SWDGE dma_gather instruction (the memory-roofline
    term), then segment-sums them on TensorE: for each chunk of 128 dst
    nodes,  OUT^T [C,128] += G_blk^T [128e,C] @ S_blk [128e,128]  where
    S is a host-precomputed one-hot(dst)*norm matrix streamed from DRAM.
    PSUM accumulates over the chunk's edge blocks.
  - ScalarE applies bias+ReLU producing the own shard of X^T for the
    next layer; shards are exchanged with an AllGather collective.
  - Everything streams in bf16 (validated: ~3e-3 rel err end to end);
    accumulation is fp32 in PSUM.
"""

import math
import os

import numpy as np
import ml_dtypes

P = 128
BF16 = ml_dtypes.bfloat16

# ---------------------------------------------------------------------------
# Host-side graph planning
# ---------------------------------------------------------------------------


def make_plan(edge_index, n_nodes, ncores=8, group_chunks=6, dims=(50, 128, 128, 128, 121)):
    """Compute the static schedule + per-core packed data from the graph."""
    src = np.concatenate([edge_index[0], np.arange(n_nodes, dtype=np.int64)])
    dst = np.concatenate([edge_index[1], np.arange(n_nodes, dtype=np.int64)])
    deg = np.bincount(dst, minlength=n_nodes).astype(np.float64)
    dinv = 1.0 / np.sqrt(deg)
    norm = (dinv[src] * dinv[dst]).astype(np.float32)

    nt = math.ceil(n_nodes / P)              # node tiles
    npad = nt * P
    t0_tiles = (nt + 1) // 2                 # low half of the gather table
    t0_rows = t0_tiles * P
    t1_rows = npad - t0_rows
    assert t0_rows < 32768 and t1_rows < 32768, "int16 gather index range"

    assert n_nodes % ncores == 0
    shard = n_nodes // ncores
    nchunks = math.ceil(shard / P)
    chunk_w = [min(P, shard - k * P) for k in range(nchunks)]

    # Bucket edges: core -> chunk -> (lo list, hi list) of (src_idx, dst_local, norm)
    core_of = dst // shard
    chunk_of = (dst % shard) // P
    is_lo = src < t0_rows

    per_core_chunk = [[([], []) for _ in range(nchunks)] for _ in range(ncores)]
    order = np.lexsort((dst,))  # group edges by dst (stable; any within-chunk order is fine)
    so, do, co, ko, lo_f, nm = (
        src[order], dst[order], core_of[order], chunk_of[order], is_lo[order], norm[order])
    for i in range(so.shape[0]):
        lists = per_core_chunk[co[i]][ko[i]]
        lst = lists[0] if lo_f[i] else lists[1]
        lst.append((so[i], do[i] % shard - ko[i] * P, nm[i]))

    # Uniform (max over cores) block counts per chunk -> identical SPMD program
    nblk_lo = [0] * nchunks
    nblk_hi = [0] * nchunks
    for k in range(nchunks):
        for c in range(ncores):
            lo, hi = per_core_chunk[c][k]
            nblk_lo[k] = max(nblk_lo[k], math.ceil(len(lo) / P))
            nblk_hi[k] = max(nblk_hi[k], math.ceil(len(hi) / P))
    nb = [nblk_lo[k] + nblk_hi[k] for k in range(nchunks)]
    totb = sum(nb)
    sblock_base = np.concatenate([[0], np.cumsum(nb)]).astype(int)

    # Gather groups of chunks; one dma_gather per (group, half)
    groups = []
    g0 = 0
    while g0 < nchunks:
        g1 = min(g0 + group_chunks, nchunks)
        ks = list(range(g0, g1))
        nb_lo_g = sum(nblk_lo[k] for k in ks)
        nb_hi_g = sum(nblk_hi[k] for k in ks)
        lo_off, hi_off, alo, ahi = {}, {}, 0, 0
        for k in ks:
            lo_off[k] = alo
            hi_off[k] = nb_lo_g + ahi
            alo += nblk_lo[k]
            ahi += nblk_hi[k]
        groups.append(dict(ks=ks, nb_lo=nb_lo_g, nb_hi=nb_hi_g,
                           lo_off=lo_off, hi_off=hi_off))
        g0 = g1
    str_max = max(g["nb_lo"] + g["nb_hi"] for g in groups)
    nb_max = max(nb)

    # gidx column layout: calls in order (g0-lo, g0-hi, g1-lo, ...)
    col_off = []
    cols = 0
    for g in groups:
        for half in (0, 1):
            n_idx = (g["nb_lo"] if half == 0 else g["nb_hi"]) * P
            col_off.append(cols)
            cols += n_idx // 16
    idxcols = cols

    # Per-core packed arrays
    gidx_all, sblk_all = [], []
    for c in range(ncores):
        gidx = np.zeros((P, idxcols), dtype=np.int16)
        sblk = np.zeros((P, totb * P), dtype=BF16)
        call = 0
        for g in groups:
            for half in (0, 1):
                idx_list = []
                for k in g["ks"]:
                    lst = per_core_chunk[c][k][half]
                    nbk = nblk_lo[k] if half == 0 else nblk_hi[k]
                    a = np.zeros(nbk * P, dtype=np.int16)
                    for j, (s, _, _) in enumerate(lst):
                        a[j] = s if half == 0 else s - t0_rows
                    idx_list.append(a)
                    # S entries for this chunk's blocks of this half
                    boff = sblock_base[k] + (0 if half == 0 else nblk_lo[k])
                    for j, (_, dl, w) in enumerate(lst):
                        bi = boff + j // P
                        sblk[j % P, bi * P + dl] = w
                arr = np.concatenate(idx_list) if idx_list else np.zeros(0, np.int16)
                n_idx = arr.shape[0]
                if n_idx:
                    wrapped = arr.reshape(-1, 16).T          # [16, n/16]
                    gidx[:, col_off[call]:col_off[call] + n_idx // 16] = (
                        np.tile(wrapped, (8, 1)))
                call += 1
        gidx_all.append(gidx)
        sblk_all.append(np.ascontiguousarray(sblk))

    return dict(
        n_nodes=n_nodes, npad=npad, nt=nt, t0_tiles=t0_tiles, t0_rows=t0_rows,
        t1_rows=t1_rows, ncores=ncores, shard=shard, nchunks=nchunks,
        chunk_w=chunk_w, nblk_lo=nblk_lo, nblk_hi=nblk_hi, nb=nb, totb=totb,
        sblock_base=sblock_base, groups=groups, str_max=str_max, nb_max=nb_max,
        col_off=col_off, idxcols=idxcols, gidx=gidx_all, sblk=sblk_all,
        dims=list(dims), dinv=dinv.astype(np.float32),
    )


# ---------------------------------------------------------------------------
# Bass program
# ---------------------------------------------------------------------------


def build_nc(plan):
    import concourse.bass as bass
    import concourse.mybir as mybir
    import concourse.tile as tile
    import concourse.tile_utils as tile_utils

    tile_utils.max_sbuf_usage = 204 * 1024  # stale 192K constant; 208K usable on trn2

    dt = mybir.dt
    nl = len(plan["dims"]) - 1
    npad, nt, shard = plan["npad"], plan["nt"], plan["shard"]
    nchunks, ncores = plan["nchunks"], plan["ncores"]
    t0_rows = plan["t0_rows"]
    cin0 = plan["dims"][0]
    dgrp = 8  # dense tiles per psum group (2 PSUM banks)

    nc = bass.Bass("TRN2", target_bir_lowering=False, debug=False,
                   num_devices=ncores)

    # I/O
    xt0 = nc.dram_tensor("xt0", [cin0, npad], dt.bfloat16, kind="ExternalInput")
    wmat = nc.dram_tensor("wmat", [nl * P, P], dt.bfloat16, kind="ExternalInput")
    bvec = nc.dram_tensor("bvec", [P, nl - 1], dt.float32, kind="ExternalInput")
    gidx = nc.dram_tensor("gidx", [P, plan["idxcols"]], dt.int16, kind="ExternalInput")
    sblk = nc.dram_tensor("sblk", [P, plan["totb"] * P], dt.bfloat16, kind="ExternalInput")
    out = nc.dram_tensor("out", [P, shard], dt.float32, kind="ExternalOutput")

    # Internal DRAM
    table = nc.dram_tensor("table", [npad, P], dt.bfloat16)
    bounce_in = nc.dram_tensor("bounce_in", [P, shard], dt.bfloat16)
    bounce_out = nc.dram_tensor("bounce_out", [ncores * P, shard], dt.bfloat16,
                                addr_space="Shared")
    tview = table.ap().rearrange("(n p) m -> p n m", p=P)  # [128, nt, 128]

    with tile.TileContext(nc) as tc:
        with (
            tc.tile_pool(name="resident", bufs=1) as rpool,
            tc.tile_pool(name="gbuf", bufs=2) as gpool,
            tc.tile_pool(name="sbuf_s", bufs=3) as spool,
            tc.tile_pool(name="hstage", bufs=3) as hpool,
            tc.tile_pool(name="ostage", bufs=2) as opool,
            tc.tile_pool(name="dense_psum", bufs=2, space="PSUM") as dppool,
            tc.tile_pool(name="sel_psum", bufs=2, space="PSUM") as sppool,
        ):
            # Resident SBUF
            xbuf = rpool.tile([P, npad], dt.bfloat16, tag="xbuf")
            xown = rpool.tile([P, shard], dt.bfloat16, tag="xown")
            gidx_sb = rpool.tile([P, plan["idxcols"]], dt.int16, tag="gidx")
            wsb = rpool.tile([P, nl * P], dt.bfloat16, tag="wsb")
            bsb = rpool.tile([P, nl - 1], dt.float32, tag="bsb")

            nc.sync.dma_start(gidx_sb[:], gidx.ap())
            for l in range(nl):
                nc.sync.dma_start(wsb[:, l * P:(l + 1) * P], wmat.ap()[l * P:(l + 1) * P, :])
            nc.sync.dma_start(bsb[:], bvec.ap())
            # zero the padded tail columns once (layers >=1 read all 128 rows)
            nc.vector.memset(xbuf[:, plan["n_nodes"]:npad], 0)
            nc.sync.dma_start(xbuf[0:cin0, :], xt0.ap())

            for l in range(nl):
                cin = plan["dims"][l] if l == 0 else P
                # ---- dense phase: H = X @ W -> gather table ----
                for g0 in range(0, nt, dgrp):
                    g1 = min(g0 + dgrp, nt)
                    w = (g1 - g0) * P
                    ph = dppool.tile([P, dgrp * P], dt.float32, tag="ph")
                    for j, t in enumerate(range(g0, g1)):
                        nc.tensor.matmul(
                            ph[:, j * P:(j + 1) * P],
                            lhsT=xbuf[0:cin, t * P:(t + 1) * P],
                            rhs=wsb[0:cin, l * P:(l + 1) * P],
                            start=True, stop=True)
                    hs = hpool.tile([P, dgrp * P], dt.bfloat16, tag="hs")
                    par = (g0 // dgrp) % 2
                    if par == 0:
                        nc.scalar.activation(hs[:, :w], ph[:, :w],
                                             mybir.ActivationFunctionType.Copy)
                        nc.scalar.dma_start(
                            tview[:, g0:g1, :],
                            hs[:, :w].rearrange("p (n m) -> p n m", m=P))
                    else:
                        nc.vector.tensor_copy(hs[:, :w], ph[:, :w])
                        nc.vector.dma_start(
                            tview[:, g0:g1, :],
                            hs[:, :w].rearrange("p (n m) -> p n m", m=P))

                # ---- gather + selection-matmul phase ----
                ost = None
                for g in plan["groups"]:
                    gt = gpool.tile([P, plan["str_max"], P], dt.bfloat16, tag="G")
                    call0 = 2 * plan["groups"].index(g)
                    if g["nb_lo"]:
                        n_idx = g["nb_lo"] * P
                        c0 = plan["col_off"][call0]
                        nc.gpsimd.dma_gather(
                            gt[:, 0:g["nb_lo"], :], table.ap()[0:t0_rows, :],
                            gidx_sb[:, c0:c0 + n_idx // 16],
                            num_idxs=n_idx, num_idxs_reg=n_idx, elem_size=P)
                    if g["nb_hi"]:
                        n_idx = g["nb_hi"] * P
                        c0 = plan["col_off"][call0 + 1]
                        nc.gpsimd.dma_gather(
                            gt[:, g["nb_lo"]:g["nb_lo"] + g["nb_hi"], :],
                            table.ap()[t0_rows:npad, :],
                            gidx_sb[:, c0:c0 + n_idx // 16],
                            num_idxs=n_idx, num_idxs_reg=n_idx, elem_size=P)
                    for k in g["ks"]:
                        nbk = plan["nb"][k]
                        base = plan["sblock_base"][k]
                        st = spool.tile([P, plan["nb_max"] * P], dt.bfloat16, tag="S")
                        nc.sync.dma_start(st[:, :nbk * P],
                                          sblk.ap()[:, base * P:(base + nbk) * P])
                        po = sppool.tile([P, P], dt.float32, tag="po")
                        bi = 0
                        for j in range(plan["nblk_lo"][k]):
                            nc.tensor.matmul(
                                po[:], lhsT=gt[:, g["lo_off"][k] + j, :],
                                rhs=st[:, bi * P:(bi + 1) * P],
                                start=(bi == 0), stop=(bi == nbk - 1))
                            bi += 1
                        for j in range(plan["nblk_hi"][k]):
                            nc.tensor.matmul(
                                po[:], lhsT=gt[:, g["hi_off"][k] + j, :],
                                rhs=st[:, bi * P:(bi + 1) * P],
                                start=(bi == 0), stop=(bi == nbk - 1))
                            bi += 1
                        wk = plan["chunk_w"][k]
                        if l < nl - 1:
                            nc.scalar.activation(
                                xown[:, k * P:k * P + wk], po[:, :wk],
                                mybir.ActivationFunctionType.Relu,
                                bias=bsb[:, l:l + 1], scale=1.0)
                        else:
                            if k % 4 == 0:
                                if ost is not None:
                                    o0 = (k - 4) * P
                                    nc.sync.dma_start(out.ap()[:, o0:o0 + 4 * P], ost[:, :4 * P])
                                ost = opool.tile([P, 4 * P], dt.float32, tag="ost")
                            nc.scalar.activation(
                                ost[:, (k % 4) * P:(k % 4) * P + wk], po[:, :wk],
                                mybir.ActivationFunctionType.Copy)
                if ost is not None:
                    k_last0 = (nchunks - 1) // 4 * 4
                    o0 = k_last0 * P
                    wlast = sum(plan["chunk_w"][k_last0:nchunks])
                    nc.sync.dma_start(out.ap()[:, o0:o0 + wlast], ost[:, :wlast])

                # ---- exchange phase ----
                if l < nl - 1:
                    nc.vector.dma_start(bounce_in.ap(), xown[:])
                    nc.gpsimd.collective_compute(
                        "AllGather", mybir.AluOpType.bypass,
                        replica_groups=[list(range(ncores))],
                        ins=[bounce_in.ap().opt()],
                        outs=[bounce_out.ap().opt()])
                    for c in range(ncores):
                        nc.sync.dma_start(
                            xbuf[:, c * shard:(c + 1) * shard],
                            bounce_out.ap()[c * P:(c + 1) * P, :])
    return nc


# ---------------------------------------------------------------------------
# Input packing / output assembly
# ---------------------------------------------------------------------------


def build_in_maps(plan, x, weights, biases):
    nl = len(plan["dims"]) - 1
    npad, cin0 = plan["npad"], plan["dims"][0]
    xt0 = np.zeros((cin0, npad), dtype=BF16)
    xt0[:, :plan["n_nodes"]] = np.asarray(x, np.float32).T.astype(BF16)
    wmat = np.zeros((nl * P, P), dtype=BF16)
    for l, w in enumerate(weights):
        w = np.asarray(w, np.float32)
        wmat[l * P:l * P + w.shape[0], :w.shape[1]] = w.astype(BF16)
    bvec = np.zeros((P, nl - 1), dtype=np.float32)
    for l in range(nl - 1):
        b = np.asarray(biases[l], np.float32)
        bvec[:b.shape[0], l] = b
    in_maps = []
    for c in range(plan["ncores"]):
        in_maps.append({
            "xt0": xt0, "wmat": wmat, "bvec": bvec,
            "gidx": plan["gidx"][c], "sblk": plan["sblk"][c],
        })
    return in_maps


def assemble_output(plan, results, biases):
    shard = plan["shard"]
    cout = plan["dims"][-1]
    full = np.zeros((P, plan["n_nodes"]), dtype=np.float32)
    for c, r in enumerate(results):
        full[:, c * shard:(c + 1) * shard] = r["out"]
    y = full[:cout, :].T + np.asarray(biases[-1], np.float32)[None, :]
    return np.ascontiguousarray(y)


LAST_EXEC_NS = None


def kernel(x, edge_index, weights, biases):
    global LAST_EXEC_NS
    from concourse import bass_utils

    x = np.asarray(x)
    edge_index = np.asarray(edge_index).astype(np.int64)
    plan = make_plan(edge_index, n_nodes=x.shape[0])
    nc = build_nc(plan)
    in_maps = build_in_maps(plan, x, weights, biases)
    trace = bool(int(os.environ.get("GCN_TRACE", "0")))
    res = bass_utils.run_bass_kernel_spmd(
        nc, in_maps, core_ids=list(range(plan["ncores"])), trace=trace)
    LAST_EXEC_NS = res.exec_time_ns
    return assemble_output(plan, res.results, biases)


# revision 22
# speedup vs baseline: 178.1336x; 178.1336x over previous
"""4-layer GCN (message passing) on 8 Trainium2 NeuronCores.

Layer: x1 = A_norm @ (x @ W) + b ; x = relu(x1). A_norm = D^-1/2 (A+I) D^-1/2.

Strategy (dst-sharded graph parallel, per the sharding hint):
  - Nodes are partitioned across the 8 cores by destination (6250 each).
  - Each core redundantly computes the full dense transform H = X @ W
    (cheap on TensorE) and writes H as a row-major bf16 gather table in
    its local DRAM.
  - Each core gathers H[src] rows for the edges pointing into its own
    dst shard with the SWDGE dma_gather instruction (the memory-roofline
    term), then segment-sums them on TensorE: for each chunk of 128 dst
    nodes,  OUT^T [C,128] += G_blk^T [128e,C] @ S_blk [128e,128]  where
    S is a host-precomputed one-hot(dst)*norm matrix streamed from DRAM.
    PSUM accumulates over the chunk's edge blocks.
  - ScalarE applies bias+ReLU producing the own shard of X^T for the
    next layer; shards are exchanged with an AllGather collective.
  - Everything streams in bf16 (validated: ~3e-3 rel err end to end);
    accumulation is fp32 in PSUM.
"""

import math
import os

import numpy as np
import ml_dtypes

P = 128
BF16 = ml_dtypes.bfloat16

# ---------------------------------------------------------------------------
# Host-side graph planning
# ---------------------------------------------------------------------------


def make_plan(edge_index, n_nodes, ncores=8, group_chunks=6, dims=(50, 128, 128, 128, 121)):
    """Compute the static schedule + per-core packed data from the graph."""
    src = np.concatenate([edge_index[0], np.arange(n_nodes, dtype=np.int64)])
    dst = np.concatenate([edge_index[1], np.arange(n_nodes, dtype=np.int64)])
    deg = np.bincount(dst, minlength=n_nodes).astype(np.float64)
    dinv = 1.0 / np.sqrt(deg)
    norm = (dinv[src] * dinv[dst]).astype(np.float32)

    nt = math.ceil(n_nodes / P)              # node tiles
    npad = nt * P
    t0_tiles = (nt + 1) // 2                 # low half of the gather table
    t0_rows = t0_tiles * P
    t1_rows = npad - t0_rows
    assert t0_rows < 32768 and t1_rows < 32768, "int16 gather index range"

    assert n_nodes % ncores == 0
    shard = n_nodes // ncores
    nchunks = math.ceil(shard / P)
    chunk_w = [min(P, shard - k * P) for k in range(nchunks)]

    # Bucket edges by (core, chunk, half); vectorized
    core_of = (dst // shard).astype(np.int64)
    chunk_of = ((dst % shard) // P).astype(np.int64)
    is_hi = (src >= t0_rows).astype(np.int64)
    gid = (core_of * nchunks + chunk_of) * 2 + is_hi     # bucket id
    n_buckets = ncores * nchunks * 2
    order = np.argsort(gid, kind="stable")
    gid_s = gid[order]
    counts = np.bincount(gid_s, minlength=n_buckets)
    starts = np.concatenate([[0], np.cumsum(counts)])
    ordinal = np.arange(gid_s.shape[0]) - starts[gid_s]  # j within bucket

    counts3 = counts.reshape(ncores, nchunks, 2)
    # Uniform (max over cores) block counts per chunk -> identical SPMD program
    nblk_arr = -(-counts3.max(axis=0) // P)              # [nchunks, 2]
    nblk_lo = nblk_arr[:, 0].tolist()
    nblk_hi = nblk_arr[:, 1].tolist()
    nb = [nblk_lo[k] + nblk_hi[k] for k in range(nchunks)]
    totb = sum(nb)
    sblock_base = np.concatenate([[0], np.cumsum(nb)]).astype(int)

    # Gather groups of chunks; one dma_gather per (group, half)
    groups = []
    g0 = 0
    while g0 < nchunks:
        g1 = min(g0 + group_chunks, nchunks)
        ks = list(range(g0, g1))
        nb_lo_g = sum(nblk_lo[k] for k in ks)
        nb_hi_g = sum(nblk_hi[k] for k in ks)
        lo_off, hi_off, alo, ahi = {}, {}, 0, 0
        for k in ks:
            lo_off[k] = alo
            hi_off[k] = nb_lo_g + ahi
            alo += nblk_lo[k]
            ahi += nblk_hi[k]
        groups.append(dict(ks=ks, nb_lo=nb_lo_g, nb_hi=nb_hi_g,
                           lo_off=lo_off, hi_off=hi_off))
        g0 = g1
    str_max = max(g["nb_lo"] + g["nb_hi"] for g in groups)
    nb_max = max(nb)

    # gidx column layout: calls in order (g0-lo, g0-hi, g1-lo, ...)
    col_off = []
    cols = 0
    for g in groups:
        for half in (0, 1):
            n_idx = (g["nb_lo"] if half == 0 else g["nb_hi"]) * P
            col_off.append(cols)
            cols += n_idx // 16
    idxcols = cols

    # Per-edge placement (vectorized):
    #   block index within chunk, call-local slot, gidx column position
    group_of_chunk = np.zeros(nchunks, dtype=np.int64)
    call_local_base = np.zeros((nchunks, 2), dtype=np.int64)  # stripe base in call
    call_of = np.zeros((nchunks, 2), dtype=np.int64)
    for gi, g in enumerate(groups):
        for k in g["ks"]:
            group_of_chunk[k] = gi
            call_of[k, 0] = 2 * gi
            call_of[k, 1] = 2 * gi + 1
            call_local_base[k, 0] = g["lo_off"][k]
            call_local_base[k, 1] = g["hi_off"][k] - g["nb_lo"]

    so = src[order]
    co = core_of[order]
    ko = chunk_of[order]
    ho = is_hi[order]
    dlo = (dst[order] % shard - ko * P).astype(np.int64)   # dst local in chunk
    nmo = norm[order]
    idx_val = (so - ho * t0_rows).astype(np.int16)
    blk = (sblock_base[ko] + ho * np.asarray(nblk_lo)[ko] + ordinal // P)
    call_idx = call_of[ko, ho]
    pos = call_local_base[ko, ho] * P + ordinal            # slot within call
    col = np.asarray(col_off)[call_idx] + pos // 16
    row16 = pos % 16

    gidx_all, sblk_all = [], []
    for c in range(ncores):
        m = co == c
        gidx = np.zeros((P, idxcols), dtype=np.int16)
        gidx[row16[m], col[m]] = idx_val[m]
        gidx = np.tile(gidx[:16], (8, 1))
        sblk = np.zeros((P, totb * P), dtype=BF16)
        sblk[ordinal[m] % P, blk[m] * P + dlo[m]] = nmo[m].astype(BF16)
        gidx_all.append(gidx)
        sblk_all.append(sblk)

    return dict(
        n_nodes=n_nodes, npad=npad, nt=nt, t0_tiles=t0_tiles, t0_rows=t0_rows,
        t1_rows=t1_rows, ncores=ncores, shard=shard, nchunks=nchunks,
        chunk_w=chunk_w, nblk_lo=nblk_lo, nblk_hi=nblk_hi, nb=nb, totb=totb,
        sblock_base=sblock_base, groups=groups, str_max=str_max, nb_max=nb_max,
        col_off=col_off, idxcols=idxcols, gidx=gidx_all, sblk=sblk_all,
        dims=list(dims), dinv=dinv.astype(np.float32),
    )


# ---------------------------------------------------------------------------
# Bass program
# ---------------------------------------------------------------------------


def build_nc(plan):
    import concourse.mybir as mybir
    import concourse.tile as tile
    import concourse.tile_utils as tile_utils
    from concourse import bacc

    tile_utils.max_sbuf_usage = 204 * 1024  # stale 192K constant; 208K usable on trn2

    dt = mybir.dt
    nl = len(plan["dims"]) - 1
    nl_emit = int(os.environ.get("GCN_NL", str(nl)))
    no_coll = bool(int(os.environ.get("GCN_NOCOLL", "0")))
    phase = os.environ.get("GCN_PHASE", "full")  # dense|gather|sel|full
    npad, nt, shard = plan["npad"], plan["nt"], plan["shard"]
    nchunks, ncores = plan["nchunks"], plan["ncores"]
    t0_rows = plan["t0_rows"]
    cin0 = plan["dims"][0]
    dgrp = 8  # dense tiles per psum group (2 PSUM banks)

    nc = bacc.Bacc("TRN2", target_bir_lowering=False, debug=False,
                   num_devices=ncores)

    # I/O
    xt0 = nc.dram_tensor("xt0", [cin0, npad], dt.bfloat16, kind="ExternalInput")
    wmat = nc.dram_tensor("wmat", [nl * P, P], dt.bfloat16, kind="ExternalInput")
    bvec = nc.dram_tensor("bvec", [P, nl - 1], dt.float32, kind="ExternalInput")
    gidx = nc.dram_tensor("gidx", [P, plan["idxcols"]], dt.int16, kind="ExternalInput")
    sblk = nc.dram_tensor("sblk", [P, plan["totb"] * P], dt.bfloat16, kind="ExternalInput")
    out = nc.dram_tensor("out", [P, shard], dt.float32, kind="ExternalOutput")

    # Internal DRAM
    table = nc.dram_tensor("table", [npad, P], dt.bfloat16)
    bounce_in = nc.dram_tensor("bounce_in", [P, shard], dt.bfloat16)
    bounce_out = nc.dram_tensor("bounce_out", [ncores * P, shard], dt.bfloat16,
                                addr_space="Shared" if ncores > 4 else "Local")
    tview = table.ap().rearrange("(n p) m -> p n m", p=P)  # [128, nt, 128]

    with tile.TileContext(nc) as tc:
        with (
            tc.tile_pool(name="resident", bufs=1) as rpool,
            tc.tile_pool(name="gbuf", bufs=2) as gpool,
            tc.tile_pool(name="sbuf_s", bufs=3) as spool,
            tc.tile_pool(name="hstage", bufs=3) as hpool,
            tc.tile_pool(name="ostage", bufs=2) as opool,
            tc.tile_pool(name="dense_psum", bufs=2, space="PSUM") as dppool,
            tc.tile_pool(name="sel_psum", bufs=2, space="PSUM") as sppool,
        ):
            # Resident SBUF
            xbuf = rpool.tile([P, npad], dt.bfloat16, tag="xbuf")
            xown = rpool.tile([P, shard], dt.bfloat16, tag="xown")
            gidx_sb = rpool.tile([P, plan["idxcols"]], dt.int16, tag="gidx")
            wsb = rpool.tile([P, nl * P], dt.bfloat16, tag="wsb")
            bsb = rpool.tile([P, nl - 1], dt.float32, tag="bsb")

            nc.sync.dma_start(gidx_sb[:], gidx.ap())
            for l in range(nl):
                nc.sync.dma_start(wsb[:, l * P:(l + 1) * P], wmat.ap()[l * P:(l + 1) * P, :])
            nc.sync.dma_start(bsb[:], bvec.ap())
            # zero the padded tail columns once (layers >=1 read all 128 rows)
            if npad > plan["n_nodes"]:
                nc.vector.memset(xbuf[:, plan["n_nodes"]:npad], 0)
            nc.sync.dma_start(xbuf[0:cin0, :], xt0.ap())

            for l in range(nl_emit):
                cin = plan["dims"][l] if l == 0 else P
                # ---- dense phase: H = X @ W -> gather table ----
                for g0 in range(0, nt, dgrp):
                    g1 = min(g0 + dgrp, nt)
                    w = (g1 - g0) * P
                    ph = dppool.tile([P, dgrp * P], dt.float32, tag="ph")
                    for j, t in enumerate(range(g0, g1)):
                        nc.tensor.matmul(
                            ph[:, j * P:(j + 1) * P],
                            lhsT=xbuf[0:cin, t * P:(t + 1) * P],
                            rhs=wsb[0:cin, l * P:(l + 1) * P],
                            start=True, stop=True)
                    hs = hpool.tile([P, dgrp * P], dt.bfloat16, tag="hs")
                    par = (g0 // dgrp) % 2
                    if par == 0:
                        nc.scalar.activation(hs[:, :w], ph[:, :w],
                                             mybir.ActivationFunctionType.Copy)
                        nc.scalar.dma_start(
                            tview[:, g0:g1, :],
                            hs[:, :w].rearrange("p (n m) -> p n m", m=P))
                    else:
                        nc.vector.tensor_copy(hs[:, :w], ph[:, :w])
                        nc.sync.dma_start(
                            tview[:, g0:g1, :],
                            hs[:, :w].rearrange("p (n m) -> p n m", m=P))

                # ---- gather + selection-matmul phase ----
                if phase == "dense":
                    continue
                ost = None
                for g in plan["groups"]:
                    gt = gpool.tile([P, plan["str_max"], P], dt.bfloat16, tag="G")
                    call0 = 2 * plan["groups"].index(g)
                    if g["nb_lo"]:
                        n_idx = g["nb_lo"] * P
                        c0 = plan["col_off"][call0]
                        nc.gpsimd.dma_gather(
                            gt[:, 0:g["nb_lo"], :], table.ap()[0:t0_rows, :],
                            gidx_sb[:, c0:c0 + n_idx // 16],
                            num_idxs=n_idx, num_idxs_reg=n_idx, elem_size=P,
                            single_packet=False)
                    if g["nb_hi"]:
                        n_idx = g["nb_hi"] * P
                        c0 = plan["col_off"][call0 + 1]
                        nc.gpsimd.dma_gather(
                            gt[:, g["nb_lo"]:g["nb_lo"] + g["nb_hi"], :],
                            table.ap()[t0_rows:npad, :],
                            gidx_sb[:, c0:c0 + n_idx // 16],
                            num_idxs=n_idx, num_idxs_reg=n_idx, elem_size=P,
                            single_packet=False)
                    if phase == "gather":
                        continue
                    for k in g["ks"]:
                        nbk = plan["nb"][k]
                        base = plan["sblock_base"][k]
                        st = spool.tile([P, plan["nb_max"] * P], dt.bfloat16, tag="S")
                        nc.sync.dma_start(st[:, :nbk * P],
                                          sblk.ap()[:, base * P:(base + nbk) * P])
                        po = sppool.tile([P, P], dt.float32, tag="po")
                        bi = 0
                        for j in range(plan["nblk_lo"][k]):
                            nc.tensor.matmul(
                                po[:], lhsT=gt[:, g["lo_off"][k] + j, :],
                                rhs=st[:, bi * P:(bi + 1) * P],
                                start=(bi == 0), stop=(bi == nbk - 1))
                            bi += 1
                        for j in range(plan["nblk_hi"][k]):
                            nc.tensor.matmul(
                                po[:], lhsT=gt[:, g["hi_off"][k] + j, :],
                                rhs=st[:, bi * P:(bi + 1) * P],
                                start=(bi == 0), stop=(bi == nbk - 1))
                            bi += 1
                        wk = plan["chunk_w"][k]
                        if l < nl - 1:
                            nc.scalar.activation(
                                xown[:, k * P:k * P + wk], po[:, :wk],
                                mybir.ActivationFunctionType.Relu,
                                bias=bsb[:, l:l + 1], scale=1.0)
                        else:
                            if k % 4 == 0:
                                if ost is not None:
                                    o0 = (k - 4) * P
                                    nc.sync.dma_start(out.ap()[:, o0:o0 + 4 * P], ost[:, :4 * P])
                                ost = opool.tile([P, 4 * P], dt.float32, tag="ost")
                            nc.scalar.activation(
                                ost[:, (k % 4) * P:(k % 4) * P + wk], po[:, :wk],
                                mybir.ActivationFunctionType.Copy)
                if ost is not None:
                    k_last0 = (nchunks - 1) // 4 * 4
                    o0 = k_last0 * P
                    wlast = sum(plan["chunk_w"][k_last0:nchunks])
                    nc.sync.dma_start(out.ap()[:, o0:o0 + wlast], ost[:, :wlast])

                # ---- exchange phase ----
                if phase in ("gather", "sel"):
                    continue
                if l < nl - 1:
                    nc.scalar.dma_start(bounce_in.ap(), xown[:])
                    if no_coll:
                        for c in range(ncores):
                            nc.sync.dma_start(
                                bounce_out.ap()[c * P:(c + 1) * P, :], bounce_in.ap())
                    else:
                        nc.gpsimd.collective_compute(
                            "AllGather", mybir.AluOpType.bypass,
                            replica_groups=[list(range(ncores))],
                            ins=[bounce_in.ap().opt()],
                            outs=[bounce_out.ap().opt()])
                    for c in range(ncores):
                        nc.sync.dma_start(
                            xbuf[:, c * shard:(c + 1) * shard],
                            bounce_out.ap()[c * P:(c + 1) * P, :])
    nc.compile()
    return nc


# ---------------------------------------------------------------------------
# Input packing / output assembly
# ---------------------------------------------------------------------------


def build_in_maps(plan, x, weights, biases):
    nl = len(plan["dims"]) - 1
    npad, cin0 = plan["npad"], plan["dims"][0]
    xt0 = np.zeros((cin0, npad), dtype=BF16)
    xt0[:, :plan["n_nodes"]] = np.asarray(x, np.float32).T.astype(BF16)
    wmat = np.zeros((nl * P, P), dtype=BF16)
    for l, w in enumerate(weights):
        w = np.asarray(w, np.float32)
        wmat[l * P:l * P + w.shape[0], :w.shape[1]] = w.astype(BF16)
    bvec = np.zeros((P, nl - 1), dtype=np.float32)
    for l in range(nl - 1):
        b = np.asarray(biases[l], np.float32)
        bvec[:b.shape[0], l] = b
    in_maps = []
    for c in range(plan["ncores"]):
        in_maps.append({
            "xt0": xt0, "wmat": wmat, "bvec": bvec,
            "gidx": plan["gidx"][c], "sblk": plan["sblk"][c],
        })
    return in_maps


def assemble_output(plan, results, biases):
    shard = plan["shard"]
    cout = plan["dims"][-1]
    full = np.zeros((P, plan["n_nodes"]), dtype=np.float32)
    for c, r in enumerate(results):
        full[:, c * shard:(c + 1) * shard] = r["out"]
    y = full[:cout, :].T + np.asarray(biases[-1], np.float32)[None, :]
    return np.ascontiguousarray(y)


LAST_EXEC_NS = None


def _run_pjrt(nc, in_maps, n_cores, time_iters=0):
    """Mirror bass2jax.run_bass_via_pjrt's multi-core path, with an optional
    steady-state wall-clock timing loop (no NTFF profiling under this axon)."""
    import time

    import jax
    import concourse.mybir as mybir
    from concourse import bass2jax
    from jax.sharding import Mesh, PartitionSpec
    from jax.experimental.shard_map import shard_map

    bass2jax.install_neuronx_cc_hook()
    partition_name = (nc.partition_id_tensor.name if nc.partition_id_tensor else None)
    in_names, out_names, out_avals, zero_outs = [], [], [], []
    for alloc in nc.m.functions[0].allocations:
        if not isinstance(alloc, mybir.MemoryLocationSet):
            continue
        name = alloc.memorylocations[0].name
        if alloc.kind == "ExternalInput":
            if name != partition_name:
                in_names.append(name)
        elif alloc.kind == "ExternalOutput":
            out_names.append(name)
            shape = tuple(alloc.tensor_shape)
            dtype = mybir.dt.np(alloc.dtype)
            out_avals.append(jax.core.ShapedArray(shape, dtype))
            zero_outs.append(np.zeros(shape, dtype))
    n_params = len(in_names)
    n_outs = len(out_avals)
    all_in_names = list(in_names) + list(out_names)
    if partition_name is not None:
        all_in_names.append(partition_name)

    def _body(*args):
        operands = list(args)
        if partition_name is not None:
            operands.append(bass2jax.partition_id_tensor())
        outs = bass2jax._bass_exec_p.bind(
            *operands,
            out_avals=tuple(out_avals),
            in_names=tuple(all_in_names),
            out_names=tuple(out_names),
            lowering_input_output_aliases=(),
            sim_require_finite=True,
            sim_require_nnan=True,
            nc=nc,
        )
        return tuple(outs)

    try:
        devices = jax.devices("neuron")[:n_cores]
    except RuntimeError:
        devices = [d for d in jax.devices() if d.platform != "cpu"][:n_cores]
    assert len(devices) == n_cores, f"need {n_cores} neuron cores"
    mesh = Mesh(np.asarray(devices), ("core",))
    in_specs = (PartitionSpec("core"),) * (n_params + n_outs)
    out_specs = (PartitionSpec("core"),) * n_outs
    mapped = shard_map(_body, mesh=mesh, in_specs=in_specs, out_specs=out_specs,
                       check_rep=False)
    donate = tuple(range(n_params, n_params + n_outs))
    sharded = jax.jit(mapped, donate_argnums=donate, keep_unused=True)
    per_core = [[np.asarray(m[name]) for name in in_names] for m in in_maps]
    concat_in = [np.concatenate([per_core[c][i] for c in range(n_cores)], axis=0)
                 for i in range(n_params)]
    concat_zeros = [np.zeros((n_cores * z.shape[0], *z.shape[1:]), z.dtype)
                    for z in zero_outs]
    out_arrs = sharded(*concat_in, *concat_zeros)
    results = [
        {name: np.asarray(out_arrs[i]).reshape(n_cores, *out_avals[i].shape)[c]
         for i, name in enumerate(out_names)}
        for c in range(n_cores)
    ]

    best_ns = None
    if time_iters > 0:
        from jax.sharding import NamedSharding
        shd = NamedSharding(mesh, PartitionSpec("core"))
        timed = jax.jit(mapped, keep_unused=True)  # no donation: reuse inputs
        t0 = time.perf_counter()
        din = [jax.device_put(a, shd) for a in concat_in]
        dzero = [jax.device_put(z, shd) for z in concat_zeros]
        jax.block_until_ready(din + dzero)
        print("  [timing] device_put: %.3fs" % (time.perf_counter() - t0))
        t0 = time.perf_counter()
        o = timed(*din, *dzero)
        jax.block_until_ready(o)
        print("  [timing] warmup call: %.3fs" % (time.perf_counter() - t0))
        times = []
        for _ in range(time_iters):
            t0 = time.perf_counter()
            o = timed(*din, *dzero)
            jax.block_until_ready(o)
            times.append(time.perf_counter() - t0)
        print("  [timing] iters:", " ".join("%.3f" % t for t in times))
        best_ns = int(min(times) * 1e9)
    return results, best_ns


def kernel(x, edge_index, weights, biases):
    global LAST_EXEC_NS

    x = np.asarray(x)
    edge_index = np.asarray(edge_index).astype(np.int64)
    plan = make_plan(edge_index, n_nodes=x.shape[0])
    nc = build_nc(plan)
    in_maps = build_in_maps(plan, x, weights, biases)
    time_iters = int(os.environ.get("GCN_TIME", "0"))
    results, best_ns = _run_pjrt(nc, in_maps, plan["ncores"], time_iters)
    LAST_EXEC_NS = best_ns
    return assemble_output(plan, results, biases)
